# revision 89
# baseline (speedup 1.0000x reference)
"""Trainium2 Bass kernel for nn_MultiHeadAttention_85229331022244.

Computation (per batch b):
  xh = x.reshape(B,T,64,16); q/k/v = per-head 64x64 projections of xh
  q,k: interleaved RoPE over the FULL 1024-dim feature axis
  scores = q @ k.T / sqrt(1024)  (single attention map over full D)
  causal softmax; y = attn @ v

Sharding: core i -> batch i//2, q-tile parity i%2 (even/odd 128-row q-tiles
interleaved between the two cores of a batch).  Every core runs an identical
program; causality differences are carried in per-core mask data.

Device layout trick: heads are reordered even-first and paired so the
projections become 8 block-diagonal 128x128 matmuls that produce K^T/Q^T
directly in [feature-on-partition, token] layout, with RoPE partner features
living in chunk c and c+4 at the same partition index.
"""

import math
from contextlib import ExitStack

import numpy as np
import ml_dtypes

import concourse.bass as bass
import concourse.mybir as mybir
import concourse.tile as tile
from concourse import bacc
from concourse.bass import ts, ds
from concourse.masks import make_identity

BF16 = ml_dtypes.bfloat16

D_MODEL = 1024
N_HEADS = 16
HEAD_D = 64
ROPE_BASE = 10000.0
GAMMA = 1.0 / math.sqrt(D_MODEL)

# head pairs per 128-row chunk; chunks 0-3 = even heads, 4-7 = odd heads
HEAD_PAIRS = [(0, 2), (4, 6), (8, 10), (12, 14), (1, 3), (5, 7), (9, 11), (13, 15)]


def _feature_perm():
    """perm[c*128 + p] = original feature index for kernel row (c, p)."""
    perm = np.zeros(1024, dtype=np.int64)
    for c, (ha, hb) in enumerate(HEAD_PAIRS):
        for p in range(128):
            h = ha if p < 64 else hb
            perm[c * 128 + p] = (p % 64) * 16 + h
    return perm


PERM = _feature_perm()
INV_PERM = np.argsort(PERM)
# y/V feature-block order: chunk cp's pair (cp, cp+4) stored adjacently
V_CHUNK_ORDER = [0, 4, 1, 5, 2, 6, 3, 7]
YPERM = PERM.reshape(8, 128)[V_CHUNK_ORDER].reshape(-1)
INV_YPERM = np.argsort(YPERM)


def _block_weights(w):
    """w: (64, 64, 16) -> (8, 128, 128) block-diag per chunk, bf16."""
    out = np.zeros((8, 128, 128), dtype=np.float32)
    for c, (ha, hb) in enumerate(HEAD_PAIRS):
        out[c, :64, :64] = w[:, :, ha]
        out[c, 64:, 64:] = w[:, :, hb]
    return out.astype(BF16)


def _rope_tables(T):
    """cos/sin tables [4, 128, T] bf16 for chunks 0-3 (and partners 4-7)."""
    p = np.arange(128)
    cos = np.zeros((4, 128, T), dtype=np.float32)
    sin = np.zeros((4, 128, T), dtype=np.float32)
    t = np.arange(T, dtype=np.float64)
    for c in range(4):
        f = (p % 64) * 8 + (2 * c + p // 64)  # [128]
        inv_freq = ROPE_BASE ** (-f.astype(np.float64) / 512.0)  # [128]
        ang = inv_freq[:, None] * t[None, :]  # [128, T]
        cos[c] = np.cos(ang).astype(np.float32)
        sin[c] = np.sin(ang).astype(np.float32)
    return cos.astype(BF16), sin.astype(BF16)


def _n_stripes(j):
    return (2 * j + 2 + 3) // 4


def _last_width(j):
    nblk = 2 * j + 2
    w = nblk - 4 * (_n_stripes(j) - 1)
    return w * 128  # 256 (j even) or 512 (j odd)


def _masks_for_core(q_tiles, NQ):
    """[NQ, 128, 512] fp32 additive masks for each q-tile's last stripe."""
    m = np.zeros((NQ, 128, 512), dtype=np.float32)
    for j, G in enumerate(q_tiles):
        k0 = 4 * (_n_stripes(j) - 1) * 128  # global key col of stripe start
        tq = G * 128 + np.arange(128)[:, None]
        tk = k0 + np.arange(512)[None, :]
        m[j] = np.where(tk <= tq, 0.0, -1e9)
    return m


def build_nc(T, NQ):
    """Build the (identical-on-all-cores) Bass program.

    T:  total key length (keys 0..T-1 resident per core)
    NQ: number of 128-row query tiles handled by this core
    Requires: max blocks = 2*(NQ-1)+2 <= T//128, T % 512 == 0, NQ % 4 == 0.
    """
    assert T % 512 == 0 and NQ % 4 == 0
    assert 2 * NQ <= T // 128
    n_kv_stripes = T // 512
    n_q_stripes = NQ * 128 // 512
    dt = mybir.dt

    nc = bacc.Bacc("TRN2", target_bir_lowering=False)
    xpT = nc.dram_tensor("xpT", [128, 8, T], dt.bfloat16, kind="ExternalInput")
    xqT = nc.dram_tensor("xqT", [128, 8, NQ * 128], dt.bfloat16,
                         kind="ExternalInput")
    w2q = nc.dram_tensor("w2q", [128, 8, 128], dt.bfloat16, kind="ExternalInput")
    w2k = nc.dram_tensor("w2k", [128, 8, 128], dt.bfloat16, kind="ExternalInput")
    w2v = nc.dram_tensor("w2v", [128, 8, 128], dt.bfloat16, kind="ExternalInput")
    csk = nc.dram_tensor("csk", [128, 8, T], dt.bfloat16, kind="ExternalInput")
    csq = nc.dram_tensor("csq", [128, 8, NQ * 128], dt.bfloat16,
                         kind="ExternalInput")
    masks = nc.dram_tensor("masks", [NQ // 2, 128, 2, 512], dt.bfloat16,
                           kind="ExternalInput")
    eb4 = nc.dram_tensor("eb4", [128, 4, 4], dt.bfloat16, kind="ExternalInput")
    y = nc.dram_tensor("y", [NQ * 128, 1024], dt.bfloat16, kind="ExternalOutput")

    with tile.TileContext(nc) as tc, ExitStack() as ctx:
        const = ctx.enter_context(tc.tile_pool(name="const", bufs=1))
        kv = ctx.enter_context(tc.tile_pool(name="kv", bufs=1))
        qpool = ctx.enter_context(tc.tile_pool(name="qpool", bufs=2))
        xpool = ctx.enter_context(tc.tile_pool(name="xpool", bufs=2))
        cspool = ctx.enter_context(tc.tile_pool(name="cspool", bufs=2))
        rtmp = ctx.enter_context(tc.tile_pool(name="rtmp", bufs=2))
        mpool = ctx.enter_context(tc.tile_pool(name="mpool", bufs=2))
        ppool = ctx.enter_context(tc.tile_pool(name="ppool", bufs=3))
        ptpool = ctx.enter_context(tc.tile_pool(name="ptpool", bufs=2))
        ypool = ctx.enter_context(tc.tile_pool(name="ypool", bufs=2))
        lpool = ctx.enter_context(tc.tile_pool(name="lpool", bufs=2))
        psum = ctx.enter_context(tc.tile_pool(name="psum", bufs=2, space="PSUM"))
        psum1 = ctx.enter_context(tc.tile_pool(name="psum1", bufs=2, space="PSUM"))

        # constants
        ident = const.tile([128, 128], dt.bfloat16, tag="ident", name="ident")
        make_identity(nc, ident)
        # eb[:, b, :]: [128, 4] one-hot column b (for per-block V column sums)
        eb = const.tile([128, 4, 4], dt.bfloat16, tag="eb", name="eb")
        nc.sync.dma_start(eb[:], eb4[:])
        # ones [32, 128] (prefix-sliced as the R-injection lhsT)
        ones32 = const.tile([32, 128], dt.bfloat16, tag="ones32", name="ones32")
        nc.vector.memset(ones32[:], 1.0)
        # per-block V column sums (bf16), written as stripes arrive
        bsmat = const.tile([32, 1024], dt.bfloat16, tag="bsmat", name="bsmat")
        wq_all = const.tile([128, 8, 128], dt.bfloat16, tag="wq", name="wq")
        wk_all = const.tile([128, 8, 128], dt.bfloat16, tag="wk", name="wk")
        wv_all = const.tile([128, 8, 128], dt.bfloat16, tag="wv", name="wv")
        nc.sync.dma_start(wq_all[:], w2q[:])
        nc.sync.dma_start(wk_all[:], w2k[:])
        nc.sync.dma_start(wv_all[:], w2v[:])
        wq_sb = [wq_all[:, c, :] for c in range(8)]
        wk_sb = [wk_all[:, c, :] for c in range(8)]
        wv_sb = [wv_all[:, c, :] for c in range(8)]

        # resident K^T in fp8: [128, subpair i, plane p, 512] per superpair —
        # plane pairs (chunk 2sp+i, 2sp+i+4) feed DoubleRow QK^T matmuls,
        # and RoPE writes plane slices for two chunk-pairs in one DVE op.
        KT = {}
        for s in range(n_kv_stripes):
            for sp in range(2):
                KT[(sp, s)] = kv.tile([128, 2, 2, 512], dt.float8e4,
                                      tag=f"kt{sp}_{s}", name=f"kt{sp}_{s}")
        V = [
            kv.tile([128, 4, 1024], dt.bfloat16, tag=f"v{s}", name=f"v{s}")
            for s in range(n_kv_stripes)
        ]
        V8 = [
            kv.tile([128, 4, 1024], dt.float8e4, tag=f"v8{s}", name=f"v8{s}")
            for s in range(n_kv_stripes)
        ]
        # Q^T streamed per 512-token stripe: tags per chunk pair
        QT = {}

        def stripe_dma(sl, x_dram, cs_dram, split=False):
            """One batched DMA each for x (8 chunks) and cos|sin of a stripe.

            split=True (warmup): issue per-superpair piece DMAs so the first
            projection can start after half the transfer (superpair sp reads
            x chunks {2sp,2sp+1,2sp+4,2sp+5} and cs planes likewise).
            """
            xall = xpool.tile([128, 8, 512], dt.bfloat16, tag="xall", name="xall")
            cs = cspool.tile([128, 8, 512], dt.bfloat16, tag="cs", name="cs")
            if split:
                for sp in range(2):
                    for half in (2 * sp, 2 * sp + 4):
                        nc.sync.dma_start(xall[:, half:half + 2, :],
                                          x_dram[:, half:half + 2, sl])
                        nc.sync.dma_start(cs[:, half:half + 2, :],
                                          cs_dram[:, half:half + 2, sl])
            else:
                nc.sync.dma_start(xall[:], x_dram[:, :, sl])
                nc.sync.dma_start(cs[:], cs_dram[:, :, sl])
            return xall, cs

        def proj_rope_super(sp, st, w_sb, out4, do_v, v_stripe):
            """One superpair (chunk-pairs 2sp and 2sp+1) over one 512-token
            stripe, split into pump units. RoPE runs on [128, 2, 512] merged
            operands (both chunk-pairs per DVE op).

            st: dict holding the stripe's staged xall/cos|sin tiles;
            out4: [128, 2, 2, 512] fp8 tile (subpair, even/odd plane, tok).
            Returns a list of thunks.
            """
            state = {}
            c0 = 2 * sp

            def u_proj():
                xall, _ = st["t"]
                ke = rtmp.tile([128, 2, 512], dt.bfloat16, tag="ke", name="ke")
                ko = rtmp.tile([128, 2, 512], dt.bfloat16, tag="ko", name="ko")
                # interleave matmul->copy per plane so the copies (which gate
                # the DVE RoPE chain) overlap the remaining matmuls
                for i in range(2):
                    pe = psum.tile([128, 512], dt.float32, tag="A", name="A")
                    po = psum.tile([128, 512], dt.float32, tag="B", name="B")
                    nc.tensor.matmul(pe[:], lhsT=w_sb[c0 + i],
                                     rhs=xall[:, c0 + i, :], start=True,
                                     stop=True)
                    nc.tensor.matmul(po[:], lhsT=w_sb[c0 + i + 4],
                                     rhs=xall[:, c0 + i + 4, :], start=True,
                                     stop=True)
                    nc.scalar.copy(ke[:, i, :], pe[:])
                    nc.scalar.copy(ko[:, i, :], po[:])
                state.update(ke=ke, ko=ko)

            def u_rope_e():
                _, cs = st["t"]
                ke, ko = state["ke"], state["ko"]
                ta = rtmp.tile([128, 2, 512], dt.bfloat16, tag="ta", name="ta")
                tb = rtmp.tile([128, 2, 512], dt.bfloat16, tag="tb", name="tb")
                # out_e = ke*cos - ko*sin (DVE: gpsimd contends for SBUF ports)
                nc.vector.tensor_mul(ta[:], ke[:], cs[:, c0:c0 + 2, :])
                nc.vector.tensor_mul(tb[:], ko[:], cs[:, 4 + c0:4 + c0 + 2, :])
                nc.vector.tensor_sub(out4[:, :, 0, :], ta[:], tb[:])

            def u_rope_o():
                _, cs = st["t"]
                ke, ko = state["ke"], state["ko"]
                ta2 = rtmp.tile([128, 2, 512], dt.bfloat16, tag="ta", name="ta")
                tb2 = rtmp.tile([128, 2, 512], dt.bfloat16, tag="tb", name="tb")
                nc.vector.tensor_mul(ta2[:], ke[:], cs[:, 4 + c0:4 + c0 + 2, :])
                nc.vector.tensor_mul(tb2[:], ko[:], cs[:, c0:c0 + 2, :])
                nc.vector.tensor_add(out4[:, :, 1, :], ta2[:], tb2[:])

            def mk_u_v(cp):
                def u_v():
                    xall, _ = st["t"]
                    xa, xb = xall[:, cp, :], xall[:, cp + 4, :]
                    v_bf, v_f8 = v_stripe
                    va = psum.tile([128, 4, 128], dt.float32, tag="A", name="VA")
                    vb = psum.tile([128, 4, 128], dt.float32, tag="B", name="VB")
                    for sub in range(4):
                        nc.tensor.matmul(
                            va[:, sub, :], lhsT=xa[:, ts(sub, 128)],
                            rhs=wv_sb[cp], start=True, stop=True,
                        )
                        nc.tensor.matmul(
                            vb[:, sub, :], lhsT=xb[:, ts(sub, 128)],
                            rhs=wv_sb[cp + 4], start=True, stop=True,
                        )
                    # V feature blocks stored in chunk order [0,4,1,5,2,6,3,7]
                    # so this pair's two blocks sit adjacently and the fp8
                    # shadow needs a single DVE cast (host unpermutes y).
                    nc.any.tensor_copy(v_bf[:, :, ds(2 * cp * 128, 128)], va[:])
                    nc.any.tensor_copy(v_bf[:, :, ds((2 * cp + 1) * 128, 128)],
                                       vb[:])
                    nc.vector.tensor_copy(v_f8[:, :, ds(2 * cp * 128, 256)],
                                          v_bf[:, :, ds(2 * cp * 128, 256)])

                return u_v

            units = [u_proj, u_rope_e, u_rope_o]
            if do_v:
                units += [mk_u_v(c0), mk_u_v(c0 + 1)]
            return units

        def kv_thunks(s, split=False):
            sl = ds(s * 512, 512)
            st = {}

            def u_dma():
                st["t"] = stripe_dma(sl, xpT, csk, split)

            def mk(sp):
                return proj_rope_super(sp, st, wk_sb,
                                       KT[(sp, s)], True, (V[s], V8[s]))

            def bs():
                # per-block column sums of V (rows 4s..4s+3 of bsmat).
                # Engines need 32-aligned partition bases, so stage at
                # partition 0 and let DMA (no alignment rule) place the rows.
                lo = psum.tile([4, 512], dt.float32, tag="A", name="BSlo")
                hi = psum.tile([4, 512], dt.float32, tag="B", name="BShi")
                for b in range(4):
                    nc.tensor.matmul(lo[:], lhsT=eb[:, b, :], rhs=V[s][:, b, 0:512],
                                     start=(b == 0), stop=(b == 3))
                    nc.tensor.matmul(hi[:], lhsT=eb[:, b, :],
                                     rhs=V[s][:, b, 512:1024],
                                     start=(b == 0), stop=(b == 3))
                bst = mpool.tile([4, 1024], dt.bfloat16, tag="bst", name="bst",
                                 bufs=1)
                nc.scalar.copy(bst[:, 0:512], lo[:])
                nc.scalar.copy(bst[:, 512:1024], hi[:])
                nc.sync.dma_start(bsmat[ds(4 * s, 4), :], bst[:])

            return [u_dma] + [u for sp in range(2) for u in mk(sp)] + [bs]

        def q_thunks(qs, split=False):
            sl = ds(qs * 512, 512)
            st = {}

            def u_dma():
                st["t"] = stripe_dma(sl, xqT, csq, split)

            def mk(sp):
                QT[(sp, qs)] = qpool.tile([128, 2, 2, 512], dt.float8e4,
                                          tag=f"qt{sp}", name=f"qt{sp}")
                return proj_rope_super(sp, st, wq_sb,
                                       QT[(sp, qs)], False, None)

            return [u_dma] + [u for sp in range(2) for u in mk(sp)]

        # producer thunks (later stripes' proj+RoPE) interleaved into q-tile
        # emission so their DVE chains overlap tensor-engine S/PV work.
        # Entries are (deadline_window, unit); producers may run ahead of
        # their deadline whenever a pump slot is free (K/V/Q are resident).
        feeds = []

        def pump():
            if feeds:
                feeds.pop(0)[1]()

        def drain(dl_max):
            while feeds and feeds[0][0] <= dl_max:
                feeds.pop(0)[1]()

        # ---- Phases B+C interleaved: Q^T stripe then its 4 q-tiles ----
        def emit_q_tile(j, mk):
            nst = _n_stripes(j)
            y_lo = psum1.tile([128, 512], dt.float32, tag="YL", name="YL")
            y_hi = psum1.tile([128, 512], dt.float32, tag="YH", name="YH")
            l_parts = lpool.tile([128, 16], dt.float32, tag="lp", name="lp")
            qs, qoff = j // 4, (j % 4) * 128

            # y = R + sum_f (e-1)*V8 + sum_e e*V, with R = column sums of V
            # over the 2j always-unmasked "f" blocks (prefix rows of bsmat).
            # The f part runs fp8 DoubleRow; the final 2 blocks (diagonal +
            # possibly fully-masked) stay bf16 so masking is exact.
            if j > 0:
                nc.tensor.matmul(y_lo[:], lhsT=ones32[0:2 * j, :],
                                 rhs=bsmat[0:2 * j, 0:512],
                                 start=True, stop=False)
                nc.tensor.matmul(y_hi[:], lhsT=ones32[0:2 * j, :],
                                 rhs=bsmat[0:2 * j, 512:1024],
                                 start=True, stop=False)

            def tail(P, s, w, last):
                # transpose + PSUM->SBUF + PV for stripe s (emitted one stripe
                # late so the tensor engine has S(s+1) to chew on meanwhile)
                nb = w // 128
                nf = nb - 2 if last else nb
                pt_ps = psum.tile([128, 4, 128], dt.bfloat16, tag="B", name="B")
                for b in range(nb):
                    nc.tensor.transpose(pt_ps[:, b, :], P[:, ts(b, 128)],
                                        ident[:])
                if nf:
                    # f^T = P^T - 1 in fp8, folded into the PSUM->SBUF move
                    ptf = ptpool.tile([128, 4, 128], dt.float8e4, tag="ptf",
                                      name="ptf")
                    nc.scalar.activation(
                        ptf[:, 0:nf, :], pt_ps[:, 0:nf, :],
                        mybir.ActivationFunctionType.Copy, bias=-1.0,
                    )
                    for bp in range(0, nf, 2):
                        blk = s * 4 + bp
                        v8 = V8[blk // 4]
                        nc.tensor.matmul(
                            y_lo[:], lhsT=ptf[:, bp:bp + 2, :],
                            rhs=v8[:, blk % 4:blk % 4 + 2, 0:512],
                            start=False, stop=False,
                            perf_mode=mybir.MatmulPerfMode.DoubleRow,
                        )
                        nc.tensor.matmul(
                            y_hi[:], lhsT=ptf[:, bp:bp + 2, :],
                            rhs=v8[:, blk % 4:blk % 4 + 2, 512:1024],
                            start=False, stop=False,
                            perf_mode=mybir.MatmulPerfMode.DoubleRow,
                        )
                if last:
                    pt = ptpool.tile([128, 2, 128], dt.bfloat16, tag="pt",
                                     name="pt")
                    nc.scalar.copy(pt[:], pt_ps[:, nf:nf + 2, :])
                    for i in range(2):
                        blk = s * 4 + nf + i
                        vs = V[blk // 4]
                        nc.tensor.matmul(y_lo[:], lhsT=pt[:, i, :],
                                         rhs=vs[:, blk % 4, 0:512],
                                         start=(j == 0 and i == 0),
                                         stop=(i == 1))
                        nc.tensor.matmul(y_hi[:], lhsT=pt[:, i, :],
                                         rhs=vs[:, blk % 4, 512:1024],
                                         start=(j == 0 and i == 0),
                                         stop=(i == 1))

            pending = None
            for s in range(nst):
                w = 512 if s < nst - 1 else _last_width(j)
                S = psum.tile([128, 512], dt.float32, tag="A", name="A")
                last = s == nst - 1
                for sp in range(2):
                    for i in range(2):
                        nc.tensor.matmul(
                            S[:, :w],
                            lhsT=QT[(sp, qs)][:, i, :, ds(qoff, 128)],
                            rhs=KT[(sp, s)][:, i, :, :w],
                            start=(sp == 0 and i == 0),
                            stop=(sp == 1 and i == 1 and not last),
                            perf_mode=mybir.MatmulPerfMode.DoubleRow,
                        )
                if last:
                    # fold the causal mask into the QK accumulation group as
                    # an identity matmul (I^T @ M = M) — keeps the vector
                    # engine (the producer-phase pacer) out of the S chain
                    nc.tensor.matmul(S[:, :w], lhsT=ident[:], rhs=mk[:, :w],
                                     start=False, stop=True)
                # emit tail(s-1) BEFORE exp(s): exp waits on the S matmuls,
                # and in-order scalar queues would head-of-line block the
                # already-runnable ptf/pt copies of the previous stripe
                if pending is not None:
                    tail(*pending, False)
                P = ppool.tile([128, 512], dt.bfloat16, tag="p", name="p")
                nc.scalar.activation(
                    P[:, :w], S[:, :w], mybir.ActivationFunctionType.Exp,
                    scale=GAMMA, accum_out=l_parts[:, ds(s, 1)],
                )
                for _ in range(4 if j < 8 else 2):
                    pump()
                pending = (P, s, w)
            tail(*pending, True)
            pump()
            pump()
            lsum = lpool.tile([128, 1], dt.float32, tag="ls", name="ls")
            linv = lpool.tile([128, 1], dt.float32, tag="li", name="li")
            nc.vector.tensor_reduce(lsum[:], l_parts[:, :nst],
                                    mybir.AxisListType.X, mybir.AluOpType.add)
            nc.vector.reciprocal(linv[:], lsum[:])
            y_sb = ypool.tile([128, 1024], dt.bfloat16, tag="y", name="y")
            # normalize on the scalar engine (per-partition scale AP) — the
            # vector engine paces the producer-heavy phase
            nc.scalar.activation(y_sb[:, 0:512], y_lo[:],
                                 mybir.ActivationFunctionType.Copy,
                                 scale=linv[:])
            nc.vector.tensor_scalar_mul(y_sb[:, 512:1024], y_hi[:], linv[:])
            nc.sync.dma_start(y[ts(j, 128), :], y_sb[:])

        # Schedule: stripe 0 (KV + Q) emitted up front — both stripe DMAs
        # first so the transfers overlap the unit emission; stripe s+1+
        # producers are enqueued as deadline-tagged thunks and pumped from
        # inside the q-tiles.
        # Units in critical-path order: tile 0's first S-matmul needs ALL of
        # QT/KT (proj+rope, both superpairs) while V/BS are only consumed
        # later (PV/R side). kv layout/sp: [proj, rope_e, rope_o, u_v, u_v].
        qs0 = q_thunks(0, split=True)
        kv0 = kv_thunks(0, split=True)
        qs0[0]()
        kv0[0]()
        krope = [kv0[2], kv0[3], kv0[7], kv0[8]]
        kv_rest = [kv0[4], kv0[5], kv0[9], kv0[10], kv0[11]]
        rest = ([qs0[1], kv0[1], qs0[4], kv0[6]]      # all 4 proj units
                + [u for pair in zip(qs0[2:4] + qs0[5:7], krope)
                   for u in pair]                      # rope, q/k interleaved
                + kv_rest)                             # V-proj + block sums
        for t in rest:
            t()
        next_kv = 1
        for s in range(n_kv_stripes):
            while next_kv <= min(s + 2, n_kv_stripes - 1):
                feeds.extend((next_kv, u) for u in kv_thunks(next_kv))
                next_kv += 1
            if s % 2 == 0 and (s + 2) // 2 < n_q_stripes:
                feeds.extend((s + 2, u) for u in q_thunks((s + 2) // 2))
            mk2 = mpool.tile([128, 2, 512], dt.bfloat16, tag="mask", name="mask")
            nc.sync.dma_start(mk2[:], masks[s])
            for j in (2 * s, 2 * s + 1):
                if j < NQ:
                    emit_q_tile(j, mk2[:, j % 2, :])
            drain(s + 1)

    nc.compile()
    return nc


# ------------------------- host side -------------------------


def prep_core_inputs(xb, w2q, w2k, w2v, cos_t, sin_t, parity, NQ, T):
    """Inputs for one core: batch slice xb (T, 1024) fp32, parity 0/1.

    Layouts are partition-major ([128, chunks, tokens]) so each stripe
    loads with a single DMA.
    """
    q_tiles = [2 * j + parity for j in range(NQ)]
    xpT = np.ascontiguousarray(
        xb.T[PERM].reshape(8, 128, T).transpose(1, 0, 2)).astype(BF16)
    cols = np.concatenate([np.arange(G * 128, (G + 1) * 128) for G in q_tiles])
    xqT = np.ascontiguousarray(xpT[:, :, cols])
    csk = np.ascontiguousarray(np.concatenate([cos_t, sin_t], axis=1))
    csq = np.ascontiguousarray(csk[:, :, cols])
    m = _masks_for_core(q_tiles, NQ).astype(BF16)
    return {
        "xpT": xpT,
        "xqT": xqT,
        "w2q": w2q,
        "w2k": w2k,
        "w2v": w2v,
        "csk": csk,
        "csq": csq,
        "masks": np.ascontiguousarray(
            m.reshape(NQ // 2, 2, 128, 512).transpose(0, 2, 1, 3)),
        "eb4": np.broadcast_to(np.eye(4, dtype=np.float32)[None, :, :],
                               (128, 4, 4)).astype(BF16),
    }


def core_model(inp, NQ):
    """Numpy model of what one core's program computes (fp32 math, for tests)."""
    T = inp["xpT"].shape[2]
    xpT = inp["xpT"].astype(np.float32)
    xqT = inp["xqT"].astype(np.float32)
    cosk = inp["cosk"].astype(np.float32)
    sink = inp["sink"].astype(np.float32)
    cosq = inp["cosq"].astype(np.float32)
    sinq = inp["sinq"].astype(np.float32)
    w2q = inp["w2q"].astype(np.float32)
    w2k = inp["w2k"].astype(np.float32)
    w2v = inp["w2v"].astype(np.float32)

    def proj_T(xT, w2):  # -> [8, 128, n]
        return np.stack([w2[c].T @ xT[c] for c in range(8)])

    def rope(zT, cos, sin):
        out = np.empty_like(zT)
        for c in range(4):
            e, o = zT[c], zT[c + 4]
            out[c] = e * cos[c] - o * sin[c]
            out[c + 4] = e * sin[c] + o * cos[c]
        return out

    kT = rope(proj_T(xpT, w2k), cosk, sink).reshape(1024, T)
    qT = rope(proj_T(xqT, w2q), cosq, sinq).reshape(1024, NQ * 128)
    v = np.concatenate([w2v[c].T @ xpT[c] for c in range(8)], axis=0).T  # [T, 1024]

    y = np.zeros((NQ * 128, 1024), dtype=np.float32)
    for j in range(NQ):
        nblk = 2 * j + 2
        q = qT[:, j * 128:(j + 1) * 128].T  # [128, 1024]
        keys = kT[:, : nblk * 128]
        S = q @ keys  # [128, nblk*128]
        mfull = inp["masks"][j]
        k0 = 4 * (_n_stripes(j) - 1) * 128
        S[:, k0:] += mfull[:, : nblk * 128 - k0]
        P = np.exp(GAMMA * S)
        y[j * 128:(j + 1) * 128] = (P @ v[: nblk * 128]) / P.sum(1, keepdims=True)
    return y


_NC_CACHE = {}
last_in_maps = None


def kernel(x, w_q, w_k, w_v):
    global last_in_maps
    from concourse.bass_utils import run_bass_kernel_spmd

    B, T, D = x.shape
    assert (B, T, D) == (4, 4096, 1024)
    NQ = 16
    x = np.asarray(x, dtype=np.float32)
    w2q = np.ascontiguousarray(
        _block_weights(np.asarray(w_q, dtype=np.float32)).transpose(1, 0, 2))
    w2k = np.ascontiguousarray(
        _block_weights(np.asarray(w_k, dtype=np.float32)).transpose(1, 0, 2))
    w2v = np.ascontiguousarray(
        _block_weights(np.asarray(w_v, dtype=np.float32)).transpose(1, 0, 2))
    cos_t, sin_t = _rope_tables(T)
    cos_t = np.ascontiguousarray(cos_t.transpose(1, 0, 2))
    sin_t = np.ascontiguousarray(sin_t.transpose(1, 0, 2))

    in_maps = []
    for core in range(8):
        b, parity = core // 2, core % 2
        in_maps.append(
            prep_core_inputs(x[b], w2q, w2k, w2v, cos_t, sin_t, parity, NQ, T)
        )
    last_in_maps = in_maps

    key = (T, NQ)
    if key not in _NC_CACHE:
        _NC_CACHE[key] = build_nc(T, NQ)
    nc = _NC_CACHE[key]

    res = run_bass_kernel_spmd(nc, in_maps, core_ids=list(range(8)))
    out = np.zeros((B, T, D), dtype=np.float32)
    for core in range(8):
        b, parity = core // 2, core % 2
        yk = res.results[core]["y"].astype(np.float32).reshape(NQ, 128, D)
        for j in range(NQ):
            G = 2 * j + parity
            out[b, G * 128:(G + 1) * 128, :] = yk[j][:, INV_YPERM]
    return out

